# revision 34
# baseline (speedup 1.0000x reference)
"""Trainium2 Bass kernel for nn_MixedOp (topk_masking, DARTS MixedOp w/ channel attention).

Strategy: data-parallel over batch (8 cores x 8 samples). Four device launches
with tiny host-side reductions between them (attention MLP, topk, BN stats):
  L1 "pool":    spatial sum/max per (sample, channel)           [reads x f32]
  L2 "main":    stage-A convs + pools + xtemp + stats           [reads xg bf16]
  L3 "sep2":    sep stage-B + sev + out_u (= x*ca unselected)   [reads xu f32]
  L4 "combine": weighted sum of the 9 branch sites              [reads sites bf16]

Perf notes vs the original baseline:
  - all intermediate branch sites stored in DRAM as bf16 (halves DMA)
  - dw+pw folding kept on TensorE for most sites, but the 7x7 depthwise taps
    are split across DVE + Pool engines as per-channel multiply-accumulate
    chains, followed by a single pointwise matmul -> balances PE/DVE/Pool.
  - pools (max/avg 3x3) run on DVE in bf16 (2x mode), scaled by ca at the end
    (ca > 0 so max commutes).
  - stats: sum rides on the Act psum->sbuf copy (accum_out), squares are a
    second Act pass.
  - sev (1x7/7x1) moved to L3 to balance PE across launches.
"""
import os
import numpy as np

import concourse.bass as bass
import concourse.mybir as mybir
import concourse.tile as tile
from concourse.bass_utils import run_bass_kernel_spmd

F32 = mybir.dt.float32
F32R = mybir.dt.float32r
BF16 = mybir.dt.bfloat16
ACTF = mybir.ActivationFunctionType
ALU = mybir.AluOpType

NCORES = 8
B, C, HH, WW = 64, 512, 32, 32
BL = B // NCORES            # samples per core
CP = 128                    # selected channels
CU = C - CP                 # unselected channels
HWF = HH * WW               # 1024
PAD = 4
WP = HH + 2 * PAD           # 40
PADF = WP * WP              # 1600
NCH = 2                     # chunks per sample (psum 512-col banks)
CHW = HWF // NCH            # 512
CROWS = HH // NCH           # 16 rows per chunk
EPS = 1e-5

_VERBOSE = os.environ.get("MIXEDOP_VERBOSE", "0") == "1"

# (name, k, pad, dil) for folded conv sites
GEOM = {"s3a": (3, 1, 1), "s5a": (5, 2, 1), "s7a": (7, 3, 1),
        "d3": (3, 2, 2), "d5": (5, 4, 2),
        "s3b": (3, 1, 1), "s5b": (5, 2, 1), "s7b": (7, 3, 1)}

# tap split for the 7x7 sites: how many of the 49 taps run on each engine
SPLIT = {"s7a": {"pe": 13, "dve": 26, "pool": 10},
         "s7b": {"pe": 16, "dve": 26, "pool": 7}}

L2_SITES = ["s3a", "s5a", "d3", "d5"]          # fully folded on PE in L2
L2_STAT_SITES = ["mp", "ap", "s3a", "s5a", "s7a", "d3", "d5"]
L3_STAT_SITES = ["s3b", "s5b", "s7b", "sv"]
L4_SITES = ["mp", "ap", "s3b", "s5b", "s7b", "d3", "d5", "sv", "xtemp"]


def _offsets(name):
    k, pad, dil = GEOM[name]
    return [(PAD - pad + (t // k) * dil, PAD - pad + (t % k) * dil)
            for t in range(k * k)]


def _win(zp, row0, col0, nrows=CROWS, ncols=WW):
    """Window AP into a padded [128, WP*WP] sbuf tile."""
    return bass.AP(tensor=zp.tensor, offset=zp.offset + row0 * WP + col0,
                   ap=[zp.ap[0], [WP, nrows], [1, ncols]])


def _interior(zp, cj=None):
    """Interior (unpadded) region of padded tile as write target [128,rows,32]."""
    r0 = PAD + (0 if cj is None else CROWS * cj)
    nr = HH if cj is None else CROWS
    return bass.AP(tensor=zp.tensor, offset=zp.offset + r0 * WP + PAD,
                   ap=[zp.ap[0], [WP, nr], [1, WW]])


def _dram_col128(h, offset=0):
    """[128] slice of a DRAM tensor as a partition-dim AP."""
    return bass.AP(tensor=h, offset=offset, ap=[[1, 128]])


def _fix_dma_waits(nc):
    """Walrus codegen accepts only ONE sync wait per instruction in this
    pipeline (setupSyncWait raises "Too many sync wait commands" for 2+).
    Tile freely emits multi-wait instructions. Fix: for every instruction
    with N>1 waits, inject N-1 single-wait Drain instructions immediately
    before it on the same engine — the engine observes each wait in order,
    so the all-of semantics is preserved exactly."""
    for bb in nc.main_func.blocks:
        insts = list(bb.instructions)
        newlist = []
        changed = False
        for ins in insts:
            si = getattr(ins, "sync_info", None)
            if si is not None and si.on_wait is not None and len(si.on_wait) > 1 \
                    and getattr(ins, "engine", None) is not None:
                waits = list(si.on_wait)
                for i, w in enumerate(waits[:-1]):
                    d = mybir.InstDrain(name=f"{ins.name}_w{i}", ins=[], outs=[])
                    d.engine = ins.engine
                    d.sync_info = mybir.SyncInfo(on_wait=[w], on_update=[])
                    newlist.append(d)
                    changed = True
                si.on_wait = [waits[-1]]
            newlist.append(ins)
        if changed:
            bb.instructions = newlist
    return nc


# ----------------------------------------------------------------- L1: pooling
def build_pool():
    nc = bass.Bass()
    x = nc.dram_tensor("x", [BL, C, HH, WW], F32, kind="ExternalInput")
    sums = nc.dram_tensor("sums", [C // 128, 128, BL], F32, kind="ExternalOutput")
    mx = nc.dram_tensor("mx", [C // 128, 128, BL], F32, kind="ExternalOutput")

    with tile.TileContext(nc) as tc:
        with (tc.tile_pool(name="xb", bufs=4) as xb,
              tc.tile_pool(name="tr", bufs=3) as tr,
              tc.tile_pool(name="st", bufs=8) as st):
            for cc in range(C // 128):
                scols = st.tile([128, BL], F32, tag="scols", name="scols")
                mcols = st.tile([128, BL], F32, tag="mcols", name="mcols")
                for s in range(BL):
                    xt = xb.tile([128, HWF], F32)
                    nc.sync.dma_start(xt, x[s, cc * 128:(cc + 1) * 128].rearrange("c h w -> c (h w)"))
                    trash = tr.tile([128, HWF], F32, tag="trash", name="trash")
                    nc.scalar.activation(trash, xt, ACTF.Copy, accum_out=scols[:, s:s + 1])
                    nc.vector.tensor_reduce(mcols[:, s:s + 1], xt, axis=mybir.AxisListType.X, op=ALU.max)
                nc.sync.dma_start(sums[cc], scols)
                nc.sync.dma_start(mx[cc], mcols)
    return nc


def _emit_chains(nc, offs7, n_pe, n_dve, n_pool, dwt, zp_list, accd_all, accp_all,
                 tmp_p, gsz=2):
    """Depthwise tap chains on DVE (scalar_tensor_tensor) and Pool (mul+add
    pairs; the real Pool ISA has no TensorScalarPtr-with-tensor form), emitted
    group-major (gsz samples at a time) so early psum groups' accumulators
    finish first while keeping a gsz-way interleave to hide dep latency."""
    for g0 in range(0, BL, gsz):
        ss = range(g0, g0 + gsz)
        for i in range(n_dve):
            dy, dx = offs7[n_pe + i]
            sc = dwt[:, i:i + 1]
            for s in ss:
                a3 = accd_all[s].rearrange("c (h w) -> c h w", h=HH)
                w = _win(zp_list[s], dy, dx, nrows=HH)
                if i == 0:
                    nc.vector.tensor_scalar_mul(a3, w, sc)
                else:
                    nc.vector.scalar_tensor_tensor(a3, w, sc, a3, op0=ALU.mult, op1=ALU.add)
        for i in range(n_pool):
            dy, dx = offs7[n_pe + n_dve + i]
            sc = dwt[:, n_dve + i:n_dve + i + 1]
            for s in ss:
                a3 = accp_all[s].rearrange("c (h w) -> c h w", h=HH)
                w = _win(zp_list[s], dy, dx, nrows=HH)
                if i == 0:
                    nc.gpsimd.tensor_scalar_mul(a3, w, sc)
                else:
                    tmp = tmp_p.tile([128, HWF], BF16, tag="ptmp", name="ptmp")
                    t3 = tmp.rearrange("c (h w) -> c h w", h=HH)
                    nc.gpsimd.tensor_scalar_mul(t3, w, sc)
                    nc.gpsimd.tensor_add(a3, a3, t3)


def _emit_site_psums(nc, ps_p, ot_p, scr_p, fwt, zp_list, nt, offs, sumc, sqc,
                     out_dram, extra_rhs=None):
    """Folded conv site: groups of 2 samples, each with a 2-bank [128,2,512]
    psum tile (ring of 4 -> two groups in flight so PE pipelines past the Act
    drains). Drain = one Act copy [128,1024] (+sum accum) -> bf16 -> DMA, and
    one Act square pass reading PSUM directly."""
    for g in range(BL // 2):
        pst = [ps_p.tile([128, NCH, CHW], F32, tag="ps", name="pst") for _ in range(2)]
        for t in range(nt):
            dy, dx = offs[t]
            for si in range(2):
                sj = g * 2 + si
                for cj in range(2):
                    nc.tensor.matmul(pst[si][:, cj, :], fwt[:, t, :],
                                     _win(zp_list[sj], CROWS * cj + dy, dx),
                                     start=(t == 0),
                                     stop=(t == nt - 1 and extra_rhs is None))
        if extra_rhs is not None:
            rhss, pwt = extra_rhs
            for si in range(2):
                sj = g * 2 + si
                for cj in range(2):
                    for ri, rgrp in enumerate(rhss):
                        nc.tensor.matmul(pst[si][:, cj, :], pwt,
                                         rgrp[sj][:, cj * CHW:(cj + 1) * CHW],
                                         start=(nt == 0 and ri == 0),
                                         stop=(ri == len(rhss) - 1))
        for si in range(2):
            sj = g * 2 + si
            pflat = pst[si].rearrange("c a b -> c (a b)")
            ot = ot_p.tile([128, HWF], BF16)
            nc.scalar.activation(ot, pflat, ACTF.Copy, accum_out=sumc[:, sj:sj + 1])
            trash = scr_p.tile([128, HWF], BF16, tag="scr2", name="trash2")
            nc.scalar.activation(trash, pflat, ACTF.Square, accum_out=sqc[:, sj:sj + 1])
            nc.sync.dma_start(out_dram[sj].rearrange("c h w -> c (h w)"), ot)


# ----------------------------------------------------------------- L2: stage A
def build_main():
    nc = bass.Bass()
    xg = nc.dram_tensor("xg", [BL, CP, HH, WW], BF16, kind="ExternalInput")
    cag = nc.dram_tensor("cag", [CP, BL], F32, kind="ExternalInput")
    ict = nc.dram_tensor("ict", [HWF], BF16, kind="ExternalInput")
    fw = {}
    for name in L2_SITES:
        k, _, _ = GEOM[name]
        fw[name] = nc.dram_tensor("fw_" + name, [k * k, CP, CP], BF16, kind="ExternalInput")
    fw["s7a"] = nc.dram_tensor("fw_s7a", [SPLIT["s7a"]["pe"], CP, CP], BF16, kind="ExternalInput")
    dwv7 = nc.dram_tensor("dwv7", [49 - SPLIT["s7a"]["pe"], CP], F32, kind="ExternalInput")
    pw7 = nc.dram_tensor("pw7", [CP, CP], BF16, kind="ExternalInput")

    xtemp = nc.dram_tensor("xtemp", [BL, CP, HH, WW], BF16, kind="ExternalOutput")
    site_out = {}
    for name in L2_STAT_SITES:
        site_out[name] = nc.dram_tensor(name, [BL, CP, HH, WW], BF16, kind="ExternalOutput")
    stats = nc.dram_tensor("stats", [CP, len(L2_STAT_SITES) * 2], F32, kind="ExternalOutput")

    spl = SPLIT["s7a"]
    n_pe, n_dve, n_pool = spl["pe"], spl["dve"], spl["pool"]
    offs7 = _offsets("s7a")

    with tile.TileContext(nc) as tc:
        with (tc.tile_pool(name="xgp", bufs=BL) as xg_p,
              tc.tile_pool(name="xtb", bufs=BL) as xtb_p,
              tc.tile_pool(name="zp", bufs=BL) as zp_p,
              tc.tile_pool(name="accd", bufs=BL) as accd_p,
              tc.tile_pool(name="accp", bufs=BL) as accp_p,
              tc.tile_pool(name="fw", bufs=3) as fw_p,
              tc.tile_pool(name="ot", bufs=4) as ot_p,
              tc.tile_pool(name="pool", bufs=2) as pool_p,
              tc.tile_pool(name="poolo", bufs=BL) as poolo_p,
              tc.tile_pool(name="scr", bufs=2) as scr_p,
              tc.tile_pool(name="st", bufs=24) as st_p,
              tc.tile_pool(name="one", bufs=1) as one_p,
              tc.tile_pool(name="ps", bufs=4, space="PSUM") as ps_p):

            # ---- constants
            cagt = one_p.tile([128, BL], F32, tag="cagt", name="cagt")
            nc.sync.dma_start(cagt, cag[:, :])
            ictt = one_p.tile([128, HWF], BF16, tag="ictt", name="ictt")
            nc.sync.dma_start(ictt, bass.AP(tensor=ict, offset=0, ap=[[0, 128], [1, HWF]]))
            ict3 = ictt.rearrange("c (h w) -> c h w", h=HH)
            dwt = one_p.tile([128, 49 - n_pe], F32, tag="dwt", name="dwt")
            nc.sync.dma_start(dwt, dwv7.rearrange("t c -> c t"))
            pwt = one_p.tile([128, CP], BF16, tag="pwt", name="pwt")
            nc.sync.dma_start(pwt, pw7[:, :])

            stat_cols = {}
            for name in L2_STAT_SITES:
                stat_cols[name] = (st_p.tile([128, BL], F32, tag="sumc", name="sumc_" + name),
                                   st_p.tile([128, BL], F32, tag="sqc", name="sqc_" + name))

            fwt_all = {}
            name0 = L2_SITES[0]
            fwt_all[name0] = fw_p.tile([128, GEOM[name0][0] ** 2, 128], BF16, tag="fw", name="fwt_" + name0)
            nc.sync.dma_start(fwt_all[name0], fw[name0].rearrange("t c o -> c t o"))

            zp_all, accd_all, accp_all = [], [], []
            xgts, xtbs = [], []
            # pass 1: zp tiles only — Act gets these out first so PE/DVE/Pool
            # tap streams start immediately
            for s in range(BL):
                xgt = xg_p.tile([128, HWF], BF16)
                xgts.append(xgt)
                nc.sync.dma_start(xgt, xg[s].rearrange("c h w -> c (h w)"))
                zp = zp_p.tile([128, PADF], BF16)
                zp_all.append(zp)
                nc.gpsimd.memset(zp, 0.0)
                nc.scalar.activation(_interior(zp), xgt.rearrange("c (h w) -> c h w", h=HH),
                                     ACTF.Relu, scale=cagt[:, s:s + 1])
            # remaining weights prefetch (after xg DMAs so zp tiles win the queue)
            for name in L2_SITES[1:]:
                k, _, _ = GEOM[name]
                fwt_all[name] = fw_p.tile([128, k * k, 128], BF16, tag="fw", name="fwt_" + name)
                nc.sync.dma_start(fwt_all[name], fw[name].rearrange("t c o -> c t o"))
            fwt7 = fw_p.tile([128, max(n_pe, 1), 128], BF16, tag="fw", name="fwt_s7a")
            nc.sync.dma_start(fwt7, fw["s7a"].rearrange("t c o -> c t o"))

            # pass 2: xtemp tiles
            for s in range(BL):
                xtb = xtb_p.tile([128, HWF], BF16, tag="xtb", name="xtb")
                xtbs.append(xtb)
                nc.scalar.activation(xtb, xgts[s], ACTF.Copy, scale=cagt[:, s:s + 1])
                nc.sync.dma_start(xtemp[s].rearrange("c h w -> c (h w)"), xtb)
            # pass 3: pools (stats deferred into Act's idle gaps between drains)
            pool_stats = []
            for s in range(BL):
                xtb = xtbs[s]
                xt3 = xtb.rearrange("c (h w) -> c h w", h=HH)

                # ---- maxpool 3x3 (separable, clipped edges) on bf16
                mW = pool_p.tile([128, HH, WW], BF16)
                nc.vector.tensor_copy(mW, xt3)
                nc.vector.tensor_max(mW[:, :, 0:WW - 1], mW[:, :, 0:WW - 1], xt3[:, :, 1:WW])
                nc.vector.tensor_max(mW[:, :, 1:WW], mW[:, :, 1:WW], xt3[:, :, 0:WW - 1])
                mp_t = poolo_p.tile([128, HH, WW], BF16)
                nc.vector.tensor_copy(mp_t, mW)
                nc.vector.tensor_max(mp_t[:, 0:HH - 1, :], mp_t[:, 0:HH - 1, :], mW[:, 1:HH, :])
                nc.vector.tensor_max(mp_t[:, 1:HH, :], mp_t[:, 1:HH, :], mW[:, 0:HH - 1, :])

                # ---- avgpool 3x3: separable sum then * invcnt (bf16)
                sW = pool_p.tile([128, HH, WW], BF16)
                nc.vector.tensor_copy(sW, xt3)
                nc.vector.tensor_add(sW[:, :, 0:WW - 1], sW[:, :, 0:WW - 1], xt3[:, :, 1:WW])
                nc.vector.tensor_add(sW[:, :, 1:WW], sW[:, :, 1:WW], xt3[:, :, 0:WW - 1])
                sH = pool_p.tile([128, HH, WW], BF16)
                nc.vector.tensor_copy(sH, sW)
                nc.vector.tensor_add(sH[:, 0:HH - 1, :], sH[:, 0:HH - 1, :], sW[:, 1:HH, :])
                nc.vector.tensor_add(sH[:, 1:HH, :], sH[:, 1:HH, :], sW[:, 0:HH - 1, :])
                ap_t = poolo_p.tile([128, HH, WW], BF16)
                nc.vector.tensor_mul(ap_t, sH, ict3)

                for name, t in (("mp", mp_t), ("ap", ap_t)):
                    pool_stats.append((name, s, t))
                    nc.sync.dma_start(site_out[name][s].rearrange("c h w -> c (h w)"), t)

            for s in range(BL):
                accd_all.append(accd_p.tile([128, HWF], BF16, name="accd"))
                accp_all.append(accp_p.tile([128, HWF], BF16, name="accp"))

            # ---- s7a depthwise chains: group-major so early groups finish first
            _emit_chains(nc, offs7, n_pe, n_dve, n_pool, dwt, zp_all, accd_all, accp_all, scr_p)

            def flush_pool_stats(nops):
                while pool_stats and nops > 0:
                    pname, ps_, pt = pool_stats.pop(0)
                    trash = scr_p.tile([128, HWF], BF16, tag="trash", name="trash")
                    nc.scalar.activation(trash, pt, ACTF.Copy, accum_out=stat_cols[pname][0][:, ps_:ps_ + 1])
                    trash2 = scr_p.tile([128, HWF], BF16, tag="trash", name="trash2")
                    nc.scalar.activation(trash2, pt, ACTF.Square, accum_out=stat_cols[pname][1][:, ps_:ps_ + 1])
                    nops -= 1

            for name in L2_SITES:
                k, _, _ = GEOM[name]
                sumc, sqc = stat_cols[name]
                _emit_site_psums(nc, ps_p, ot_p, scr_p, fwt_all[name], zp_all, k * k,
                                 _offsets(name), sumc, sqc, site_out[name])
                flush_pool_stats(4)

            # ---- s7a: folded subset + pointwise on the two dw accumulators
            sumc, sqc = stat_cols["s7a"]
            _emit_site_psums(nc, ps_p, ot_p, scr_p, fwt7, zp_all, n_pe,
                             offs7[:n_pe], sumc, sqc, site_out["s7a"],
                             extra_rhs=([accd_all, accp_all], pwt[:, :]))
            flush_pool_stats(99)

            # ---- finalize stats
            stout = st_p.tile([128, len(L2_STAT_SITES) * 2], F32, tag="stout", name="stout")
            for si, name in enumerate(L2_STAT_SITES):
                sumc, sqc = stat_cols[name]
                nc.vector.tensor_reduce(stout[:, si * 2:si * 2 + 1], sumc, axis=mybir.AxisListType.X, op=ALU.add)
                nc.vector.tensor_reduce(stout[:, si * 2 + 1:si * 2 + 2], sqc, axis=mybir.AxisListType.X, op=ALU.add)
            nc.sync.dma_start(stats[:, :], stout)
    return nc


# ----------------------------------------------------------------- L3: stage B + sev + out_u
def build_sep2():
    nc = bass.Bass()
    zin = {}
    for zname in ("s3a", "s5a", "s7a"):
        zin[zname] = nc.dram_tensor(zname, [BL, CP, HH, WW], BF16, kind="ExternalInput")
    bn1 = nc.dram_tensor("bn1", [3, CP, 2], F32, kind="ExternalInput")  # scale, shift
    xg = nc.dram_tensor("xg", [BL, CP, HH, WW], BF16, kind="ExternalInput")
    cag = nc.dram_tensor("cag", [CP, BL], F32, kind="ExternalInput")
    xu = nc.dram_tensor("xu", [BL, CU, HH, WW], F32, kind="ExternalInput")
    cau = nc.dram_tensor("cau", [CU // 128, 128, BL], F32, kind="ExternalInput")
    fw2 = {}
    for name in ("s3b", "s5b"):
        k, _, _ = GEOM[name]
        fw2[name] = nc.dram_tensor("fw2_" + name, [k * k, CP, CP], BF16, kind="ExternalInput")
    fw2["s7b"] = nc.dram_tensor("fw2_s7b", [SPLIT["s7b"]["pe"], CP, CP], BF16, kind="ExternalInput")
    dwv7 = nc.dram_tensor("dwv7b", [49 - SPLIT["s7b"]["pe"], CP], F32, kind="ExternalInput")
    pw7 = nc.dram_tensor("pw7b", [CP, CP], BF16, kind="ExternalInput")
    w17 = nc.dram_tensor("w17", [7, CP, CP], BF16, kind="ExternalInput")
    w71 = nc.dram_tensor("w71", [7, CP, CP], BF16, kind="ExternalInput")

    zout = {}
    for name in ("s3b", "s5b", "s7b", "sv"):
        zout[name] = nc.dram_tensor(name, [BL, CP, HH, WW], BF16, kind="ExternalOutput")
    out_u = nc.dram_tensor("out_u", [BL, CU, HH, WW], F32, kind="ExternalOutput")
    stats = nc.dram_tensor("stats", [CP, len(L3_STAT_SITES) * 2], F32, kind="ExternalOutput")

    spl = SPLIT["s7b"]
    n_pe, n_dve, n_pool = spl["pe"], spl["dve"], spl["pool"]
    offs7 = _offsets("s7b")

    with tile.TileContext(nc) as tc:
        with (tc.tile_pool(name="z1", bufs=4) as z1_p,
              tc.tile_pool(name="zp", bufs=8) as zp_p,
              tc.tile_pool(name="zp7", bufs=BL) as zp7_p,
              tc.tile_pool(name="accd", bufs=BL) as accd_p,
              tc.tile_pool(name="accp", bufs=BL) as accp_p,
              tc.tile_pool(name="xu", bufs=3) as xu_p,
              tc.tile_pool(name="ou", bufs=3) as ou_p,
              tc.tile_pool(name="fw", bufs=3) as fw_p,
              tc.tile_pool(name="ot", bufs=6) as ot_p,
              tc.tile_pool(name="scr", bufs=4) as scr_p,
              tc.tile_pool(name="st", bufs=16) as st_p,
              tc.tile_pool(name="one", bufs=1) as one_p,
              tc.tile_pool(name="ps", bufs=4, space="PSUM") as ps_p):

            cagt = one_p.tile([128, BL], F32, tag="cagt", name="cagt")
            nc.sync.dma_start(cagt, cag[:, :])
            dwt = one_p.tile([128, 49 - n_pe], F32, tag="dwt", name="dwt")
            nc.sync.dma_start(dwt, dwv7.rearrange("t c -> c t"))
            pwt = one_p.tile([128, CP], BF16, tag="pwt", name="pwt")
            nc.sync.dma_start(pwt, pw7[:, :])
            w17t = one_p.tile([128, 7, 128], BF16, tag="w17", name="w17t")
            nc.sync.dma_start(w17t, w17.rearrange("t c o -> c t o"))
            w71t = one_p.tile([128, 7, 128], BF16, tag="w71", name="w71t")
            nc.sync.dma_start(w71t, w71.rearrange("t c o -> c t o"))
            caut = {}
            for cc in range(CU // 128):
                caut[cc] = one_p.tile([128, BL], F32, tag=f"caut{cc}", name=f"caut{cc}")
                nc.sync.dma_start(caut[cc], cau[cc])
            fwt_all = {}
            for name in ("s3b", "s5b"):
                k, _, _ = GEOM[name]
                fwt_all[name] = fw_p.tile([128, k * k, 128], BF16, tag="fw", name="fwt_" + name)
                nc.sync.dma_start(fwt_all[name], fw2[name].rearrange("t c o -> c t o"))
            fwt7 = fw_p.tile([128, max(n_pe, 1), 128], BF16, tag="fw", name="fwt_s7b")
            nc.sync.dma_start(fwt7, fw2["s7b"].rearrange("t c o -> c t o"))

            stat_cols = {}
            for name in L3_STAT_SITES:
                stat_cols[name] = (st_p.tile([128, BL], F32, tag="sumc", name="sumc_" + name),
                                   st_p.tile([128, BL], F32, tag="sqc", name="sqc_" + name))

            def make_zp2(aname, si, pool, memset=True):
                bncol = st_p.tile([128, 2], F32, tag="bncol", name="bncol_" + aname)
                nc.sync.dma_start(bncol, bn1[si])
                tiles = []
                for s in range(BL):
                    z1t = z1_p.tile([128, HWF], BF16)
                    nc.sync.dma_start(z1t, zin[aname][s].rearrange("c h w -> c (h w)"))
                    zp = pool.tile([128, PADF], BF16, tag="zpv", name="zp2")
                    if memset:
                        nc.gpsimd.memset(zp, 0.0)
                    nc.scalar.activation(_interior(zp), z1t.rearrange("c (h w) -> c h w", h=HH),
                                         ACTF.Relu, bias=bncol[:, 1:2], scale=bncol[:, 0:1])
                    tiles.append(zp)
                return tiles

            # ---- padded-tile preps, carefully ordered: zpv feeds sev's PE
            # stream, zps7[0..1] feeds the first chain group; the remaining
            # zps7 preps are deferred so sev's upad drains aren't starved
            bncol7 = st_p.tile([128, 2], F32, tag="bncol", name="bncol_s7a")
            nc.sync.dma_start(bncol7, bn1[2])
            zpv_all, zps7 = [], []

            def prep_zpv(s):
                xgt = xu_p.tile([128, HWF], BF16, tag="xgt", name="xgt")
                nc.sync.dma_start(xgt, xg[s].rearrange("c h w -> c (h w)"))
                zpv = zp_p.tile([128, PADF], BF16, tag="zpv", name="zpv")
                zpv_all.append(zpv)
                nc.gpsimd.memset(zpv, 0.0)
                nc.scalar.activation(_interior(zpv), xgt.rearrange("c (h w) -> c h w", h=HH),
                                     ACTF.Relu, scale=cagt[:, s:s + 1])

            def prep_zp7(s):
                z1t = z1_p.tile([128, HWF], BF16)
                nc.sync.dma_start(z1t, zin["s7a"][s].rearrange("c h w -> c (h w)"))
                zp7 = zp7_p.tile([128, PADF], BF16, name="zp7")
                zps7.append(zp7)
                nc.gpsimd.memset(zp7, 0.0)
                nc.scalar.activation(_interior(zp7), z1t.rearrange("c (h w) -> c h w", h=HH),
                                     ACTF.Relu, bias=bncol7[:, 1:2], scale=bncol7[:, 0:1])

            prep_zpv(0)
            prep_zpv(1)
            prep_zp7(0)
            prep_zp7(1)
            for s in range(2, BL):
                prep_zpv(s)
            accd_all, accp_all = [], []
            for s in range(BL):
                accd_all.append(accd_p.tile([128, HWF], BF16, name="accd"))
                accp_all.append(accp_p.tile([128, HWF], BF16, name="accp"))

            # ---- sev: 1x7 then 7x1 on PE
            sumc, sqc = stat_cols["sv"]
            upads = []
            for s in range(BL):
                if s == 2:
                    for s7 in range(2, BL):
                        prep_zp7(s7)
                pst1 = ps_p.tile([128, NCH, CHW], F32, tag="ps", name="pst1")
                for t in range(7):
                    for cj in range(2):
                        nc.tensor.matmul(pst1[:, cj, :], w17t[:, t, :],
                                         _win(zpv_all[s], CROWS * cj + PAD, PAD - 3 + t),
                                         start=(t == 0), stop=(t == 6))
                upad = zp_p.tile([128, PADF], BF16, tag="zpv", name="upad")
                upads.append(upad)
                nc.scalar.activation(_interior(upad),
                                     pst1.rearrange("c a (h w) -> c (a h) w", w=WW), ACTF.Copy)
            _emit_chains(nc, offs7, n_pe, n_dve, n_pool, dwt, zps7, accd_all, accp_all, scr_p)

            for s in range(BL):
                pst2 = ps_p.tile([128, NCH, CHW], F32, tag="ps", name="pst2")
                for t in range(7):
                    for cj in range(2):
                        nc.tensor.matmul(pst2[:, cj, :], w71t[:, t, :],
                                         _win(upads[s], CROWS * cj + PAD - 3 + t, PAD),
                                         start=(t == 0), stop=(t == 6))
                pflat = pst2.rearrange("c a b -> c (a b)")
                ot = ot_p.tile([128, HWF], BF16)
                nc.scalar.activation(ot, pflat, ACTF.Copy, accum_out=sumc[:, s:s + 1])
                trash = scr_p.tile([128, HWF], BF16, tag="scr2", name="trash2")
                nc.scalar.activation(trash, pflat, ACTF.Square, accum_out=sqc[:, s:s + 1])
                nc.sync.dma_start(zout["sv"][s].rearrange("c h w -> c (h w)"), ot)

            # ---- s3b, s5b fully folded
            for si, name in enumerate(("s3b", "s5b")):
                k, _, _ = GEOM[name]
                zps = make_zp2(name[:-1] + "a", si, zp_p, memset=False)
                sumc, sqc = stat_cols[name]
                _emit_site_psums(nc, ps_p, ot_p, scr_p, fwt_all[name], zps, k * k,
                                 _offsets(name), sumc, sqc, zout[name])

            # ---- s7b psums: folded subset + pointwise on the dw accumulators
            sumc, sqc = stat_cols["s7b"]
            _emit_site_psums(nc, ps_p, ot_p, scr_p, fwt7, zps7, n_pe,
                             offs7[:n_pe], sumc, sqc, zout["s7b"],
                             extra_rhs=([accd_all, accp_all], pwt[:, :]))

            # ---- out_u = ca * x on unselected channels (Act slack at the end)
            for s in range(BL):
                for cc in range(CU // 128):
                    xut = xu_p.tile([128, HWF], F32)
                    nc.sync.dma_start(xut, xu[s, cc * 128:(cc + 1) * 128].rearrange("c h w -> c (h w)"))
                    out = ou_p.tile([128, HWF], F32)
                    nc.scalar.activation(out, xut, ACTF.Copy, scale=caut[cc][:, s:s + 1])
                    nc.sync.dma_start(out_u[s, cc * 128:(cc + 1) * 128].rearrange("c h w -> c (h w)"), out)

            # ---- finalize stats
            stout = st_p.tile([128, len(L3_STAT_SITES) * 2], F32, tag="stout", name="stout")
            for si, name in enumerate(L3_STAT_SITES):
                sumc, sqc = stat_cols[name]
                nc.vector.tensor_reduce(stout[:, si * 2:si * 2 + 1], sumc, axis=mybir.AxisListType.X, op=ALU.add)
                nc.vector.tensor_reduce(stout[:, si * 2 + 1:si * 2 + 2], sqc, axis=mybir.AxisListType.X, op=ALU.add)
            nc.sync.dma_start(stats[:, :], stout)
    return nc


# ----------------------------------------------------------------- L4: combine
def build_combine():
    nc = bass.Bass()
    sites = {}
    for name in L4_SITES:
        sites[name] = nc.dram_tensor(name, [BL, CP, HH, WW], BF16, kind="ExternalInput")
    diag = nc.dram_tensor("diag", [len(L4_SITES), CP, CP], BF16, kind="ExternalInput")
    brow = nc.dram_tensor("brow", [CP], F32, kind="ExternalInput")
    temp1 = nc.dram_tensor("temp1", [BL, CP, HH, WW], BF16, kind="ExternalOutput")

    ns = len(L4_SITES)
    with tile.TileContext(nc) as tc:
        with (tc.tile_pool(name="one", bufs=1) as one_p,
              tc.tile_pool(name="sin", bufs=2 * ns) as sin_p,
              tc.tile_pool(name="ot", bufs=4) as ot_p,
              tc.tile_pool(name="ps", bufs=4, space="PSUM") as ps_p):
            diagt = one_p.tile([128, ns, 128], BF16)
            nc.sync.dma_start(diagt, diag.rearrange("s c o -> c s o"))
            brt = one_p.tile([128, 1], F32)
            nc.sync.dma_start(brt, _dram_col128(brow))
            for s in range(BL):
                stiles = []
                for ni, name in enumerate(L4_SITES):
                    t = sin_p.tile([128, HWF], BF16, tag="sin", name="sin_t")
                    eng = nc.sync if ni % 2 == 0 else nc.scalar
                    eng.dma_start(t, sites[name][s].rearrange("c h w -> c (h w)"))
                    stiles.append(t)
                pst = ps_p.tile([128, NCH, CHW], F32, tag="ps", name="pst")
                for cj in range(2):
                    for si in range(ns):
                        nc.tensor.matmul(pst[:, cj, :], diagt[:, si, :],
                                         stiles[si][:, cj * CHW:(cj + 1) * CHW],
                                         start=(si == 0), stop=(si == ns - 1))
                ot = ot_p.tile([128, HWF], BF16)
                nc.scalar.activation(ot, pst.rearrange("c a b -> c (a b)"), ACTF.Identity, bias=brt[:, 0:1])
                nc.sync.dma_start(temp1[s].rearrange("c h w -> c (h w)"), ot)
    return nc


# ----------------------------------------------------------------- host side
_CACHE = {}


def _get(name, builder):
    if name not in _CACHE:
        _CACHE[name] = builder()
    return _CACHE[name]


def _sigmoid(v):
    return (1.0 / (1.0 + np.exp(-v.astype(np.float32), dtype=np.float32))).astype(np.float32)


def _run(nc, in_maps, label):
    if not getattr(nc, "_dma_waits_fixed", False):
        _fix_dma_waits(nc)
        nc._dma_waits_fixed = True
    trace = os.environ.get("BASS_TRACE", "0") == "1"
    res = run_bass_kernel_spmd(nc, in_maps, core_ids=list(range(NCORES)), trace=trace)
    if res.exec_time_ns is not None:
        _EXEC_NS.append((label, res.exec_time_ns))
    return res.results


_EXEC_NS = []


def _fold_taps(dw, pw, tap_idx=None):
    """dw [CP,1,k,k], pw [CP,CP,1,1] -> lhsT per tap [T, c, o] (bf16)."""
    import ml_dtypes
    k = dw.shape[2]
    taps = range(k * k) if tap_idx is None else tap_idx
    pwT = pw[:, :, 0, 0].T.astype(np.float32)          # [c, o]
    out = np.empty((len(list(taps)), CP, CP), np.float32)
    for i, t in enumerate(range(k * k) if tap_idx is None else tap_idx):
        out[i] = pwT * dw[:, 0, t // k, t % k][:, None]
    return np.ascontiguousarray(out).astype(ml_dtypes.bfloat16)


def kernel(**inputs):
    import ml_dtypes
    x = np.asarray(inputs["x"], np.float32)
    weights = np.asarray(inputs["weights"], np.float32)
    weights_all = np.asarray(inputs["weights_all"], np.float32)
    w_fc1 = np.asarray(inputs["w_fc1"], np.float32)
    w_fc2 = np.asarray(inputs["w_fc2"], np.float32)

    _EXEC_NS.clear()

    shards = [x[c * BL:(c + 1) * BL] for c in range(NCORES)]

    # ---------------- L1: pooling
    nc1 = _get("pool", build_pool)
    res1 = _run(nc1, [{"x": np.ascontiguousarray(s)} for s in shards], "L1")
    avg = np.concatenate([r["sums"].reshape(C, BL).T for r in res1], 0) / np.float32(HWF)
    mxv = np.concatenate([r["mx"].reshape(C, BL).T for r in res1], 0)

    # ---------------- host: channel attention + topk
    pooled = np.concatenate([avg, mxv], 1).astype(np.float32)       # [B, 2C]
    y = pooled @ w_fc1.T                                             # [B, 10]
    A = weights_all.T @ weights_all                                  # [10, 10]
    y = np.maximum(y @ A.T, 0.0).astype(np.float32)
    ca = _sigmoid(y @ w_fc2.T)                                       # [B, C]
    slist = ca.sum(0, dtype=np.float32)
    order = np.argsort(-slist, kind="stable")
    idx = order[:CP].astype(np.int64)
    uidx = order[CP:].astype(np.int64)

    xg = np.ascontiguousarray(x[:, idx]).astype(ml_dtypes.bfloat16)  # [B, CP, H, W]
    cag = np.ascontiguousarray(ca[:, idx])
    xu = np.ascontiguousarray(x[:, uidx])                            # [B, CU, H, W] f32
    cau = np.ascontiguousarray(ca[:, uidx])

    spl2, spl3 = SPLIT["s7a"], SPLIT["s7b"]
    offs_all = list(range(49))
    dw7a = np.asarray(inputs["sep7_dw1"], np.float32)
    pw7a = np.asarray(inputs["sep7_pw1"], np.float32)
    dw7b = np.asarray(inputs["sep7_dw2"], np.float32)
    pw7b = np.asarray(inputs["sep7_pw2"], np.float32)

    fw_in = {
        "fw_s3a": _fold_taps(np.asarray(inputs["sep3_dw1"], np.float32), np.asarray(inputs["sep3_pw1"], np.float32)),
        "fw_s5a": _fold_taps(np.asarray(inputs["sep5_dw1"], np.float32), np.asarray(inputs["sep5_pw1"], np.float32)),
        "fw_d3": _fold_taps(np.asarray(inputs["dil3_dw"], np.float32), np.asarray(inputs["dil3_pw"], np.float32)),
        "fw_d5": _fold_taps(np.asarray(inputs["dil5_dw"], np.float32), np.asarray(inputs["dil5_pw"], np.float32)),
        "fw_s7a": _fold_taps(dw7a, pw7a, offs_all[:spl2["pe"]]),
    }
    dwv7 = np.ascontiguousarray(
        dw7a[:, 0].reshape(CP, 49).T[spl2["pe"]:]).astype(np.float32)       # [39, CP]
    pw7 = np.ascontiguousarray(pw7a[:, :, 0, 0].T).astype(ml_dtypes.bfloat16)  # [c, o]

    w17 = np.ascontiguousarray(
        np.asarray(inputs["w_1x7"], np.float32)[:, :, 0, :].transpose(2, 1, 0)).astype(ml_dtypes.bfloat16)
    w71 = np.ascontiguousarray(
        np.asarray(inputs["w_7x1"], np.float32)[:, :, :, 0].transpose(2, 1, 0)).astype(ml_dtypes.bfloat16)

    # avgpool inverse-count map (count_include_pad=False)
    cnt = np.zeros((HH, WW), np.float32)
    for h in range(HH):
        for w in range(WW):
            cnt[h, w] = (min(h + 1, HH - 1) - max(h - 1, 0) + 1) * (min(w + 1, WW - 1) - max(w - 1, 0) + 1)
    ict = (1.0 / cnt).reshape(-1).astype(ml_dtypes.bfloat16)

    # ---------------- L2
    nc2 = _get("main", build_main)
    in_maps = []
    for c in range(NCORES):
        m = {"xg": np.ascontiguousarray(xg[c * BL:(c + 1) * BL]),
             "cag": np.ascontiguousarray(cag[c * BL:(c + 1) * BL].T),
             "ict": ict, "dwv7": dwv7, "pw7": pw7}
        m.update(fw_in)
        in_maps.append(m)
    res2 = _run(nc2, in_maps, "L2")

    xtemp = np.concatenate([r["xtemp"] for r in res2], 0)
    stats2 = np.sum([r["stats"].astype(np.float64) for r in res2], axis=0)
    stats2 = stats2.T.reshape(len(L2_STAT_SITES), 2, CP)
    site_data = {name: np.concatenate([r[name] for r in res2], 0) for name in L2_STAT_SITES}

    n_el = B * HWF
    bn = {}
    for si, name in enumerate(L2_STAT_SITES):
        mean = (stats2[si, 0] / n_el).astype(np.float32)
        var = (stats2[si, 1] / n_el - (stats2[si, 0] / n_el) ** 2).astype(np.float32)
        scale = (1.0 / np.sqrt(var + np.float32(EPS))).astype(np.float32)
        bn[name] = (scale, (-mean * scale).astype(np.float32))

    # ---------------- L3
    nc3 = _get("sep2", build_sep2)
    bn1 = np.stack([np.stack(bn[n], axis=1) for n in ("s3a", "s5a", "s7a")]).astype(np.float32)
    fw2_in = {
        "fw2_s3b": _fold_taps(np.asarray(inputs["sep3_dw2"], np.float32), np.asarray(inputs["sep3_pw2"], np.float32)),
        "fw2_s5b": _fold_taps(np.asarray(inputs["sep5_dw2"], np.float32), np.asarray(inputs["sep5_pw2"], np.float32)),
        "fw2_s7b": _fold_taps(dw7b, pw7b, offs_all[:spl3["pe"]]),
    }
    dwv7b = np.ascontiguousarray(
        dw7b[:, 0].reshape(CP, 49).T[spl3["pe"]:]).astype(np.float32)
    pw7b_ = np.ascontiguousarray(pw7b[:, :, 0, 0].T).astype(ml_dtypes.bfloat16)
    in_maps = []
    for c in range(NCORES):
        m = {"s3a": np.ascontiguousarray(site_data["s3a"][c * BL:(c + 1) * BL]),
             "s5a": np.ascontiguousarray(site_data["s5a"][c * BL:(c + 1) * BL]),
             "s7a": np.ascontiguousarray(site_data["s7a"][c * BL:(c + 1) * BL]),
             "bn1": bn1,
             "xg": np.ascontiguousarray(xg[c * BL:(c + 1) * BL]),
             "cag": np.ascontiguousarray(cag[c * BL:(c + 1) * BL].T),
             "xu": np.ascontiguousarray(xu[c * BL:(c + 1) * BL]),
             "cau": np.ascontiguousarray(cau[c * BL:(c + 1) * BL].T.reshape(CU // 128, 128, BL)),
             "dwv7b": dwv7b, "pw7b": pw7b_, "w17": w17, "w71": w71}
        m.update(fw2_in)
        in_maps.append(m)
    res3 = _run(nc3, in_maps, "L3")
    out_u = np.concatenate([r["out_u"] for r in res3], 0)
    stats3 = np.sum([r["stats"].astype(np.float64) for r in res3], axis=0)
    stats3 = stats3.T.reshape(len(L3_STAT_SITES), 2, CP)
    for si, name in enumerate(L3_STAT_SITES):
        mean = (stats3[si, 0] / n_el).astype(np.float32)
        var = (stats3[si, 1] / n_el - (stats3[si, 0] / n_el) ** 2).astype(np.float32)
        scale = (1.0 / np.sqrt(var + np.float32(EPS))).astype(np.float32)
        bn[name] = (scale, (-mean * scale).astype(np.float32))
        site_data[name] = np.concatenate([r[name] for r in res3], 0)
    site_data["xtemp"] = xtemp

    # ---------------- L4: weighted combine
    wmap = {"mp": weights[1], "ap": weights[2], "s3b": weights[4], "s5b": weights[5],
            "s7b": weights[6], "d3": weights[7], "d5": weights[8], "sv": weights[9]}
    diag = np.zeros((len(L4_SITES), CP, CP), np.float32)
    brow = np.zeros(CP, np.float32)
    for si, name in enumerate(L4_SITES):
        if name == "xtemp":
            coef = np.full(CP, weights[3], np.float32)
        else:
            scale, shift = bn[name]
            coef = wmap[name] * scale
            brow += wmap[name] * shift
        np.fill_diagonal(diag[si], coef)

    nc4 = _get("combine", build_combine)
    in_maps = []
    for c in range(NCORES):
        m = {name: np.ascontiguousarray(site_data[name][c * BL:(c + 1) * BL]) for name in L4_SITES}
        m["diag"] = diag.astype(ml_dtypes.bfloat16)
        m["brow"] = brow
        in_maps.append(m)
    res4 = _run(nc4, in_maps, "L4")
    temp1 = np.concatenate([r["temp1"] for r in res4], 0).astype(np.float32)

    out = np.empty((B, C, HH, WW), np.float32)
    out[:, uidx] = out_u
    out[:, idx] = temp1
    if _EXEC_NS and _VERBOSE:
        for label, ns in _EXEC_NS:
            print(f"  {label}: {ns} ns")
    return out


def last_exec_times():
    return list(_EXEC_NS)
